# revision 2
# baseline (speedup 1.0000x reference)
"""Trainium2 Bass kernel for the CoLL co-occurrence layer.

Math (per image):
    scale = 8/(max(x)-min(x)+1e-8)   (global over the whole batch)
    u     = (x - xmin)*scale ;  idx = clip(floor(u), 0, 7)
    y(p)  = sum_q w[q] * x(p+q) * co[idx_p, idx_q]   over 3x3 neighborhoods q

Reformulation (staircase basis, bf16 matmul pipeline):
    g_j(p) = 1[u(p) >= j]                       j = 1..7
    basis planes: h_0 = x, h_j = x*g_j          (bf16; h_j exact product since g is 0/1)
    v_i = x * co[i, idx] = sum_j A[i,j] h_j     with A[i,j] = co[i,j]-co[i,j-1]
    V_i = conv3x3(v_i, w)                       (SAME, zero pad)
    y(p) = V_{idx_p}(p)                         via chain of predicated copies on g_i

Mapping (one image per NeuronCore, [h=128 partitions, (w,c)=8192 free]):
  - global min/max via a 2-float AllReduce(max) of (-min, max).
  - the 8x8 A-mix runs on the PE with the packed layout partition=(8r+j) over
    16-row h-groups: one DMA packs h_all[16g:16g+16,:,:] -> psi[128,:], one
    matmul with stationary kron(I16, A^T) produces v packed as (8r+i), one DMA
    unpacks back to natural v planes.  x rides as basis plane 0, so the mix
    emits v_i = x*rho_i directly (no separate multiply, no bias).
  - conv along h via tridiagonal band matmuls (contraction over h_in), conv
    along w via +-C free-dim offsets of the zero-padded, EXT-wide v tiles.
  - all matmul operands bf16 (4x PE throughput vs f32); PSUM accumulates f32.
  - engine split: u + h_j + select on DVE, g_j on GPSIMD, x->bf16 and PSUM
    evacuation on ACT, pack/unpack/y DMAs on the SP (sync) HWDGE queue.
"""

from contextlib import ExitStack

import numpy as np

import concourse.bass as bass
import concourse.tile as tile
from concourse import mybir
from concourse.tile_rust import add_dep_helper

F32 = mybir.dt.float32
BF16 = mybir.dt.bfloat16
AX = mybir.AxisListType
OP = mybir.AluOpType
ACTF = mybir.ActivationFunctionType

N, H, W, C = 8, 128, 128, 64
NB = 8
N_CORES = 8
Fd = W * C
Cd = C


def build_tables(co, w):
    """Host-side weight construction (f32; converted to bf16 on-chip).

    wmix = kron(I16, A^T): stationary for the packed mix matmul.  With moving
    operand psi[8r+j, col] = h_j[16g+r, col] it yields out[8r+i] =
    sum_j A[i,j] h_j = v_i (packed).
    band[dw][hi, ho] = w[dh, dw] at hi = ho+dh-1: tridiagonal h-conv per
    w-offset, accumulated over dw in PSUM.
    """
    co = np.asarray(co, np.float32)
    w = np.asarray(w, np.float32)
    A = co - np.concatenate([np.zeros((NB, 1), np.float32), co[:, :-1]], axis=1)
    wmix = np.kron(np.eye(16, dtype=np.float32), A.T)          # [128, 128]
    band = np.zeros((3, 128, 128), np.float32)  # [dw, h_in, h_out]
    for dw in range(3):
        for ho in range(128):
            for dh in range(3):
                hi = ho + dh - 1
                if 0 <= hi < 128:
                    band[dw, hi, ho] = w[dh, dw]
    return wmix, band


def build_bass(n_cores=N_CORES, FC=1024, reps=1):
    """Per-core Bass module; same program on every core, collective min/max
    when n_cores > 1. reps>1 wraps the main pipeline in a For_i for wall-clock
    HW timing."""
    EXT = FC + 2 * Cd
    nchunk = Fd // FC
    assert Fd % FC == 0

    from concourse.bacc import Bacc
    nc = Bacc()
    x_d = nc.declare_dram_parameter("x", [H, Fd], F32, isOutput=False)
    wmix_d = nc.declare_dram_parameter("wmix", [128, 128], F32, isOutput=False)
    band_d = nc.declare_dram_parameter("band", [3, 128, 128], F32, isOutput=False)
    y_d = nc.declare_dram_parameter("y", [H, Fd], F32, isOutput=True)
    cc_in = nc.dram_tensor("cc_in", [2], F32)
    if n_cores > 1:
        cc_out = nc.dram_tensor("cc_out", [2], F32, addr_space="Shared")

    with tile.TileContext(nc) as tc, ExitStack() as ctx:
        consts = ctx.enter_context(tc.tile_pool(name="consts", bufs=1))
        work = ctx.enter_context(tc.tile_pool(name="work", bufs=2))
        gpool = ctx.enter_context(tc.tile_pool(name="gpool", bufs=2))
        hpool = ctx.enter_context(tc.tile_pool(name="hpool", bufs=2))
        psip = ctx.enter_context(tc.tile_pool(name="psip", bufs=3))
        vkpool = ctx.enter_context(tc.tile_pool(name="vkpool", bufs=3))
        vpool = ctx.enter_context(tc.tile_pool(name="vpool", bufs=2))
        ypool = ctx.enter_context(tc.tile_pool(name="ypool", bufs=2))
        mixps = ctx.enter_context(tc.tile_pool(name="mixps", bufs=4, space="PSUM"))
        convps = ctx.enter_context(tc.tile_pool(name="convps", bufs=3, space="PSUM"))

        # ---- constants / inputs resident in SBUF ----
        xpad = consts.tile([128, Fd + 2 * Cd], F32)
        nc.gpsimd.memset(xpad[:, 0:Cd], 0.0)
        nc.gpsimd.memset(xpad[:, Cd + Fd:], 0.0)
        nc.sync.dma_start(out=xpad[:, Cd:Cd + Fd], in_=x_d[:, :])
        wmix_f = consts.tile([128, 128], F32)
        nc.sync.dma_start(out=wmix_f, in_=wmix_d[:, :])
        wmix = consts.tile([128, 128], BF16)
        nc.vector.tensor_copy(wmix, wmix_f)
        band_f = consts.tile([128, 3, 128], F32)
        nc.sync.dma_start(out=band_f, in_=band_d[:, :, :].rearrange("d i o -> i d o"))
        band = consts.tile([128, 3, 128], BF16)
        nc.vector.tensor_copy(band, band_f)

        # ---- global min/max -> scale (op-order identical to the reference) ----
        mn = consts.tile([128, 1], F32)
        mx = consts.tile([128, 1], F32)
        nc.vector.tensor_reduce(mn, xpad[:, Cd:Cd + Fd], axis=AX.X, op=OP.min)
        nc.vector.tensor_reduce(mx, xpad[:, Cd:Cd + Fd], axis=AX.X, op=OP.max)
        tmp = consts.tile([128, 2], F32)
        nc.vector.tensor_scalar_mul(tmp[:, 0:1], mn, -1.0)   # (-min, max)
        nc.vector.tensor_copy(tmp[:, 1:2], mx)
        red = consts.tile([1, 2], F32)
        nc.gpsimd.tensor_reduce(red, tmp, axis=AX.C, op=OP.max)
        pair = consts.tile([128, 2], F32)
        dma_in = nc.sync.dma_start(out=cc_in[:], in_=red)
        src = cc_in
        prev = dma_in
        if n_cores > 1:
            cc = nc.gpsimd.collective_compute(
                "AllReduce", OP.max,
                replica_groups=[list(range(n_cores))],
                ins=[cc_in.ap().opt()], outs=[cc_out.ap().opt()],
            )
            add_dep_helper(cc.ins, dma_in.ins, True, "cc waits dram write")
            src = cc_out
            prev = cc
        bcast = bass.AP(tensor=src.ap().tensor, offset=0, ap=[[0, 128], [1, 2]])
        dma_back = nc.sync.dma_start(out=pair[:, :], in_=bcast)
        add_dep_helper(dma_back.ins, prev.ins, True, "bcast waits dram ready")
        negxmin = pair[:, 0:1]
        gmax = pair[:, 1:2]

        rng = consts.tile([128, 1], F32)
        nc.vector.tensor_tensor(rng, gmax, negxmin, op=OP.add)   # xmax - xmin
        dd = consts.tile([128, 1], F32)
        nc.vector.tensor_scalar_add(dd, rng, float(np.float32(1e-8)))
        recip = consts.tile([128, 1], F32)
        nc.vector.reciprocal(recip, dd)
        scale = consts.tile([128, 1], F32)
        nc.vector.tensor_scalar_mul(scale, recip, 8.0)

        # ---- main streamed pipeline over free-dim chunks ----
        def chunk_pipeline(ci):
            cs = ci * FC  # xpad col cs .. cs+EXT covers data cols [cs-Cd, cs+FC+Cd)
            u = work.tile([128, EXT], F32, tag="u")
            nc.vector.tensor_scalar(u, xpad[:, cs:cs + EXT], negxmin, scale,
                                    op0=OP.add, op1=OP.mult)
            # g_j = (u >= j) as bf16 0/1 (mask + staircase factor), on GPSIMD
            g = gpool.tile([128, 7, EXT], BF16, tag="g")
            for j in range(1, 8):
                nc.gpsimd.tensor_scalar(g[:, j - 1, :], u, float(j), None,
                                        op0=OP.is_ge)
            # basis planes: h_all[:,0,:] = bf16(x) via ACT, h_j = x_bf*g_j (exact)
            h_all = hpool.tile([128, 8, EXT], BF16, tag="h")
            nc.scalar.activation(h_all[:, 0, :], xpad[:, cs:cs + EXT], ACTF.Copy)
            for j in range(1, 8):
                nc.vector.tensor_tensor(h_all[:, j, :], h_all[:, 0, :],
                                        g[:, j - 1, :], op=OP.mult)

            # per 16-row h-group: pack -> mix matmul -> evacuate -> unpack
            v_all = vpool.tile([128, 8, EXT], BF16, tag="v")
            spans = [(s, min(EXT, s + 512)) for s in range(0, EXT, 512)]
            for grp in range(8):
                psi = psip.tile([128, EXT], BF16, tag="psi")
                nc.sync.dma_start(out=psi, in_=h_all[16 * grp:16 * grp + 16, :, :])
                v_pk = vkpool.tile([128, EXT], BF16, tag="vpk")
                for s, e in spans:
                    pm = mixps.tile([128, e - s], F32, tag="pm")
                    nc.tensor.matmul(pm, wmix, psi[:, s:e], start=True, stop=True)
                    # evacuate PSUM -> bf16 on ACT (otherwise idle)
                    nc.scalar.activation(v_pk[:, s:e], pm, ACTF.Copy)
                nc.sync.dma_start(out=v_all[16 * grp:16 * grp + 16, :, :], in_=v_pk)

            # conv + select per 512-col output sub-chunk
            y_t = ypool.tile([128, FC], F32, tag="y")
            for sub in range(0, FC, 512):
                sw = min(512, FC - sub)
                for i in range(NB):
                    vt = convps.tile([128, sw], F32, tag="vt")
                    for dw in range(3):
                        nc.tensor.matmul(
                            vt, band[:, dw, :],
                            v_all[:, i, sub + dw * Cd:sub + dw * Cd + sw],
                            start=(dw == 0), stop=(dw == 2))
                    if i == 0:
                        nc.scalar.activation(y_t[:, sub:sub + sw], vt, ACTF.Copy)
                    else:
                        # overwrite where g_i != 0 (ascending i => y = V_idx)
                        mask = g[:, i - 1, Cd + sub:Cd + sub + sw].bitcast(
                            mybir.dt.uint16)
                        nc.vector.copy_predicated(y_t[:, sub:sub + sw], mask, vt)
            nc.sync.dma_start(out=y_d[:, cs:cs + FC], in_=y_t)

        if reps == 1:
            for ci in range(nchunk):
                chunk_pipeline(ci)
        else:
            with tc.For_i(0, reps, 1):
                for ci in range(nchunk):
                    chunk_pipeline(ci)
    nc.finalize()
    return nc


_CACHE = {}


def _run(x, co_matrix, w_spatial, trace=False):
    x = np.ascontiguousarray(np.asarray(x, np.float32))
    wmix, band = build_tables(co_matrix, w_spatial)
    n, h, w_, c = x.shape
    assert (n, h, w_, c) == (N, H, W, C), (n, h, w_, c)

    from concourse.bass_utils import run_bass_kernel_spmd

    key = "full"
    if key not in _CACHE:
        _CACHE[key] = build_bass(n_cores=N_CORES, FC=1024)
    nc = _CACHE[key]

    in_maps = []
    for core in range(N_CORES):
        in_maps.append({
            "x": x[core].reshape(H, W * C),
            "wmix": wmix,
            "band": band,
        })
    res = run_bass_kernel_spmd(nc, in_maps, core_ids=list(range(N_CORES)),
                               trace=trace)
    out = np.stack([res.results[i]["y"].reshape(H, W, C) for i in range(N_CORES)], 0)
    return out, res


def kernel(x, co_matrix, w_spatial):
    return _run(x, co_matrix, w_spatial)[0]


def run_traced(x, co_matrix, w_spatial):
    _, res = _run(x, co_matrix, w_spatial, trace=True)
    return res.exec_time_ns


# revision 5
# speedup vs baseline: 1.5969x; 1.5969x over previous
"""Trainium2 Bass kernel for the CoLL co-occurrence layer.

Math (per image):
    scale = 8/(max(x)-min(x)+1e-8)   (global over the whole batch)
    u     = (x - xmin)*scale ;  idx = clip(floor(u), 0, 7)
    y(p)  = sum_q w[q] * x(p+q) * co[idx_p, idx_q]   over 3x3 neighborhoods q

Reformulation (staircase basis, bf16 matmul pipeline):
    g_j(p) = 1[u(p) >= j]                       j = 1..7
    basis planes: h_0 = x, h_j = x*g_j          (bf16; h_j exact product since g is 0/1)
    v_i = x * co[i, idx] = sum_j A[i,j] h_j     with A[i,j] = co[i,j]-co[i,j-1]
    V_i = conv3x3(v_i, w)                       (SAME, zero pad)
    y(p) = V_{idx_p}(p)                         via chain of predicated copies on g_i

Mapping (one image per NeuronCore, [h=128 partitions, (w,c)=8192 free]):
  - global min/max via a 2-float AllReduce(max) of (-min, max).
  - the 8x8 A-mix runs on the PE with the packed layout partition=(8r+j) over
    16-row h-groups: one DMA packs h_all[16g:16g+16,:,:] -> psi[128,:], one
    matmul with stationary kron(I16, A^T) produces v packed as (8r+i), one DMA
    unpacks back to natural v planes.  x rides as basis plane 0, so the mix
    emits v_i = x*rho_i directly (no separate multiply, no bias).
  - conv along h via tridiagonal band matmuls (contraction over h_in), conv
    along w via +-C free-dim offsets of the zero-padded, EXT-wide v tiles.
  - all matmul operands bf16 (4x PE throughput vs f32); PSUM accumulates f32.
  - engine split: u + h_j + select on DVE, g_j on GPSIMD, x->bf16 and PSUM
    evacuation on ACT, pack/unpack/y DMAs on the SP (sync) HWDGE queue.
"""

from contextlib import ExitStack

import numpy as np

import concourse.bass as bass
import concourse.tile as tile
from concourse import mybir
from concourse.tile_rust import add_dep_helper

F32 = mybir.dt.float32
BF16 = mybir.dt.bfloat16
AX = mybir.AxisListType
OP = mybir.AluOpType
ACTF = mybir.ActivationFunctionType

N, H, W, C = 8, 128, 128, 64
NB = 8
N_CORES = 8
Fd = W * C
Cd = C


def build_tables(co, w):
    """Host-side weight construction (f32; converted to bf16 on-chip).

    wmix = kron(I16, A^T): stationary for the packed mix matmul.  With moving
    operand psi[8r+j, col] = h_j[16g+r, col] it yields out[8r+i] =
    sum_j A[i,j] h_j = v_i (packed).
    band[dw][hi, ho] = w[dh, dw] at hi = ho+dh-1: tridiagonal h-conv per
    w-offset, accumulated over dw in PSUM.
    """
    co = np.asarray(co, np.float32)
    w = np.asarray(w, np.float32)
    A = co - np.concatenate([np.zeros((NB, 1), np.float32), co[:, :-1]], axis=1)
    wmix = np.kron(np.eye(16, dtype=np.float32), A.T)          # [128, 128]
    band = np.zeros((3, 128, 128), np.float32)  # [dw, h_in, h_out]
    for dw in range(3):
        for ho in range(128):
            for dh in range(3):
                hi = ho + dh - 1
                if 0 <= hi < 128:
                    band[dw, hi, ho] = w[dh, dw]
    return wmix, band


def build_bass(n_cores=N_CORES, FC=1024, reps=1):
    """Per-core Bass module; same program on every core, collective min/max
    when n_cores > 1. reps>1 wraps the main pipeline in a For_i for wall-clock
    HW timing."""
    EXT = FC + 2 * Cd
    nchunk = Fd // FC
    assert Fd % FC == 0

    from concourse.bacc import Bacc
    nc = Bacc()
    x_d = nc.declare_dram_parameter("x", [H, Fd], F32, isOutput=False)
    wmix_d = nc.declare_dram_parameter("wmix", [128, 128], F32, isOutput=False)
    band_d = nc.declare_dram_parameter("band", [3, 128, 128], F32, isOutput=False)
    y_d = nc.declare_dram_parameter("y", [H, Fd], F32, isOutput=True)
    cc_in = nc.dram_tensor("cc_in", [2], F32)
    if n_cores > 1:
        cc_out = nc.dram_tensor("cc_out", [2], F32, addr_space="Shared")

    with tile.TileContext(nc) as tc, ExitStack() as ctx:
        consts = ctx.enter_context(tc.tile_pool(name="consts", bufs=1))
        work = ctx.enter_context(tc.tile_pool(name="work", bufs=2))
        gpool = ctx.enter_context(tc.tile_pool(name="gpool", bufs=2))
        hpool = ctx.enter_context(tc.tile_pool(name="hpool", bufs=2))
        psip = ctx.enter_context(tc.tile_pool(name="psip", bufs=8))
        vkpool = ctx.enter_context(tc.tile_pool(name="vkpool", bufs=8))
        vpool = ctx.enter_context(tc.tile_pool(name="vpool", bufs=2))
        ypool = ctx.enter_context(tc.tile_pool(name="ypool", bufs=2))
        mixps = ctx.enter_context(tc.tile_pool(name="mixps", bufs=4, space="PSUM"))
        convps = ctx.enter_context(tc.tile_pool(name="convps", bufs=3, space="PSUM"))

        # ---- constants / inputs resident in SBUF ----
        xpad = consts.tile([128, Fd + 2 * Cd], F32)
        nc.gpsimd.memset(xpad[:, 0:Cd], 0.0)
        nc.gpsimd.memset(xpad[:, Cd + Fd:], 0.0)
        nc.sync.dma_start(out=xpad[:, Cd:Cd + Fd], in_=x_d[:, :])
        wmix_f = consts.tile([128, 128], F32)
        nc.sync.dma_start(out=wmix_f, in_=wmix_d[:, :])
        wmix = consts.tile([128, 128], BF16)
        nc.vector.tensor_copy(wmix, wmix_f)
        band_f = consts.tile([128, 3, 128], F32)
        nc.sync.dma_start(out=band_f, in_=band_d[:, :, :].rearrange("d i o -> i d o"))
        band = consts.tile([128, 3, 128], BF16)
        nc.vector.tensor_copy(band, band_f)

        # ---- global min/max -> scale (op-order identical to the reference) ----
        mn = consts.tile([128, 1], F32)
        mx = consts.tile([128, 1], F32)
        nc.vector.tensor_reduce(mn, xpad[:, Cd:Cd + Fd], axis=AX.X, op=OP.min)
        nc.vector.tensor_reduce(mx, xpad[:, Cd:Cd + Fd], axis=AX.X, op=OP.max)
        tmp = consts.tile([128, 2], F32)
        nc.vector.tensor_scalar_mul(tmp[:, 0:1], mn, -1.0)   # (-min, max)
        nc.vector.tensor_copy(tmp[:, 1:2], mx)
        red = consts.tile([1, 2], F32)
        nc.gpsimd.tensor_reduce(red, tmp, axis=AX.C, op=OP.max)
        pair = consts.tile([128, 2], F32)
        dma_in = nc.sync.dma_start(out=cc_in[:], in_=red)
        src = cc_in
        prev = dma_in
        if n_cores > 1:
            cc = nc.gpsimd.collective_compute(
                "AllReduce", OP.max,
                replica_groups=[list(range(n_cores))],
                ins=[cc_in.ap().opt()], outs=[cc_out.ap().opt()],
            )
            add_dep_helper(cc.ins, dma_in.ins, True, "cc waits dram write")
            src = cc_out
            prev = cc
        bcast = bass.AP(tensor=src.ap().tensor, offset=0, ap=[[0, 128], [1, 2]])
        dma_back = nc.sync.dma_start(out=pair[:, :], in_=bcast)
        add_dep_helper(dma_back.ins, prev.ins, True, "bcast waits dram ready")
        negxmin = pair[:, 0:1]
        gmax = pair[:, 1:2]

        rng = consts.tile([128, 1], F32)
        nc.vector.tensor_tensor(rng, gmax, negxmin, op=OP.add)   # xmax - xmin
        dd = consts.tile([128, 1], F32)
        nc.vector.tensor_scalar_add(dd, rng, float(np.float32(1e-8)))
        recip = consts.tile([128, 1], F32)
        nc.vector.reciprocal(recip, dd)
        scale = consts.tile([128, 1], F32)
        nc.vector.tensor_scalar_mul(scale, recip, 8.0)

        # ---- main streamed pipeline over free-dim chunks ----
        def chunk_pipeline(ci):
            cs = ci * FC  # xpad col cs .. cs+EXT covers data cols [cs-Cd, cs+FC+Cd)
            u = work.tile([128, EXT], F32, tag="u")
            nc.vector.tensor_scalar(u, xpad[:, cs:cs + EXT], negxmin, scale,
                                    op0=OP.add, op1=OP.mult)
            # g_j = (u >= j) as bf16 0/1 (mask + staircase factor), on GPSIMD
            g = gpool.tile([128, 7, EXT], BF16, tag="g")
            for j in range(1, 8):
                nc.gpsimd.tensor_scalar(g[:, j - 1, :], u, float(j), None,
                                        op0=OP.is_ge)
            # basis planes: h_all[:,0,:] = bf16(x) via ACT, h_j = x_bf*g_j (exact)
            h_all = hpool.tile([128, 8, EXT], BF16, tag="h")
            nc.scalar.activation(h_all[:, 0, :], xpad[:, cs:cs + EXT], ACTF.Copy)
            for j in range(1, 8):
                nc.vector.tensor_tensor(h_all[:, j, :], h_all[:, 0, :],
                                        g[:, j - 1, :], op=OP.mult)

            # per 16-row h-group: pack -> mix matmul -> evacuate -> unpack.
            # packs go on the SP HWDGE queue, unpacks (which wait on the PSUM
            # evacuation) on the ACT HWDGE queue: HWDGE queues are in-order, so
            # a waiting unpack at the head must not block later packs.
            v_all = vpool.tile([128, 8, EXT], BF16, tag="v")
            spans = [(s, min(EXT, s + 512)) for s in range(0, EXT, 512)]
            psis = []
            for grp in range(8):
                psi = psip.tile([128, EXT], BF16, tag="psi")
                nc.sync.dma_start(out=psi, in_=h_all[16 * grp:16 * grp + 16, :, :])
                psis.append(psi)
            for grp in range(8):
                v_pk = vkpool.tile([128, EXT], BF16, tag="vpk")
                for s, e in spans:
                    pm = mixps.tile([128, e - s], F32, tag="pm")
                    nc.tensor.matmul(pm, wmix, psis[grp][:, s:e], start=True,
                                     stop=True)
                    # evacuate PSUM -> bf16 on ACT (otherwise idle)
                    nc.scalar.activation(v_pk[:, s:e], pm, ACTF.Copy)
                nc.scalar.dma_start(out=v_all[16 * grp:16 * grp + 16, :, :],
                                    in_=v_pk)

            # conv + select per 512-col output sub-chunk
            y_t = ypool.tile([128, FC], F32, tag="y")
            for sub in range(0, FC, 512):
                sw = min(512, FC - sub)
                for i in range(NB):
                    vt = convps.tile([128, sw], F32, tag="vt")
                    for dw in range(3):
                        nc.tensor.matmul(
                            vt, band[:, dw, :],
                            v_all[:, i, sub + dw * Cd:sub + dw * Cd + sw],
                            start=(dw == 0), stop=(dw == 2))
                    if i == 0:
                        nc.scalar.activation(y_t[:, sub:sub + sw], vt, ACTF.Copy)
                    else:
                        # overwrite where g_i != 0 (ascending i => y = V_idx)
                        mask = g[:, i - 1, Cd + sub:Cd + sub + sw].bitcast(
                            mybir.dt.uint16)
                        nc.vector.copy_predicated(y_t[:, sub:sub + sw], mask, vt)
            nc.scalar.dma_start(out=y_d[:, cs:cs + FC], in_=y_t)

        if reps == 1:
            for ci in range(nchunk):
                chunk_pipeline(ci)
        else:
            with tc.For_i(0, reps, 1):
                for ci in range(nchunk):
                    chunk_pipeline(ci)
    nc.finalize()
    return nc


_CACHE = {}


def _run(x, co_matrix, w_spatial, trace=False):
    x = np.ascontiguousarray(np.asarray(x, np.float32))
    wmix, band = build_tables(co_matrix, w_spatial)
    n, h, w_, c = x.shape
    assert (n, h, w_, c) == (N, H, W, C), (n, h, w_, c)

    from concourse.bass_utils import run_bass_kernel_spmd

    key = "full"
    if key not in _CACHE:
        _CACHE[key] = build_bass(n_cores=N_CORES, FC=1024)
    nc = _CACHE[key]

    in_maps = []
    for core in range(N_CORES):
        in_maps.append({
            "x": x[core].reshape(H, W * C),
            "wmix": wmix,
            "band": band,
        })
    res = run_bass_kernel_spmd(nc, in_maps, core_ids=list(range(N_CORES)),
                               trace=trace)
    out = np.stack([res.results[i]["y"].reshape(H, W, C) for i in range(N_CORES)], 0)
    return out, res


def kernel(x, co_matrix, w_spatial):
    return _run(x, co_matrix, w_spatial)[0]


def run_traced(x, co_matrix, w_spatial):
    _, res = _run(x, co_matrix, w_spatial, trace=True)
    return res.exec_time_ns


# revision 9
# speedup vs baseline: 4.4447x; 2.7834x over previous
"""Trainium2 Bass kernel for the CoLL co-occurrence layer.

Math (per image):
    scale = 8/(max(x)-min(x)+1e-8)   (global over the whole batch)
    u     = (x - xmin)*scale ;  idx = clip(floor(u), 0, 7)
    y(p)  = sum_q w[q] * x(p+q) * co[idx_p, idx_q]   over 3x3 neighborhoods q

Reformulation (staircase basis, bf16 matmul pipeline):
    g_j(p) = 1[u(p) >= j]                       j = 1..7
    basis planes: h_0 = x, h_j = x*g_j          (bf16; h_j exact product since g is 0/1)
    v_i = x * co[i, idx] = sum_j A[i,j] h_j     with A[i,j] = co[i,j]-co[i,j-1]
    V_i = conv3x3(v_i, w)                       (SAME, zero pad)
    y(p) = V_{idx_p}(p)                         via chain of predicated copies on g_i

Mapping (one image per NeuronCore, [h=128 partitions, (w,c)=8192 free]):
  - global min/max via a 2-float AllReduce(max) of (-min, max).
  - the 8x8 A-mix runs on the PE with the packed layout partition=(8r+j) over
    16-row h-groups: one DMA packs h_all[16g:16g+16,:,:] -> psi[128,:], one
    matmul with stationary kron(I16, A^T) produces v packed as (8r+i), one DMA
    unpacks back to natural v planes.  x rides as basis plane 0, so the mix
    emits v_i = x*rho_i directly (no separate multiply, no bias).
  - conv along h via tridiagonal band matmuls (contraction over h_in), conv
    along w via +-C free-dim offsets of the zero-padded, EXT-wide v tiles.
  - all matmul operands bf16 (4x PE throughput vs f32); PSUM accumulates f32.
  - engine split: u + h_j + select on DVE, g_j on GPSIMD, x->bf16 and PSUM
    evacuation on ACT, pack/unpack/y DMAs on the SP (sync) HWDGE queue.
"""

from contextlib import ExitStack

import numpy as np

import concourse.bass as bass
import concourse.tile as tile
from concourse import mybir
from concourse.tile_rust import add_dep_helper

F32 = mybir.dt.float32
BF16 = mybir.dt.bfloat16
AX = mybir.AxisListType
OP = mybir.AluOpType
ACTF = mybir.ActivationFunctionType

N, H, W, C = 8, 128, 128, 64
NB = 8
N_CORES = 8
Fd = W * C
Cd = C


def build_tables(co, w):
    """Host-side weight construction (f32; converted to bf16 on-chip).

    wmix = kron(I16, A^T): stationary for the packed mix matmul.  With moving
    operand psi[8r+j, col] = h_j[16g+r, col] it yields out[8r+i] =
    sum_j A[i,j] h_j = v_i (packed).
    band[dw][hi, ho] = w[dh, dw] at hi = ho+dh-1: tridiagonal h-conv per
    w-offset, accumulated over dw in PSUM.
    """
    co = np.asarray(co, np.float32)
    w = np.asarray(w, np.float32)
    A = co - np.concatenate([np.zeros((NB, 1), np.float32), co[:, :-1]], axis=1)
    wmix = np.kron(np.eye(16, dtype=np.float32), A.T)          # [128, 128]
    band = np.zeros((3, 128, 128), np.float32)  # [dw, h_in, h_out]
    for dw in range(3):
        for ho in range(128):
            for dh in range(3):
                hi = ho + dh - 1
                if 0 <= hi < 128:
                    band[dw, hi, ho] = w[dh, dw]
    return wmix, band


def build_bass(n_cores=N_CORES, FC=1024, reps=1):
    """Per-core Bass module; same program on every core, collective min/max
    when n_cores > 1. reps>1 wraps the main pipeline in a For_i for wall-clock
    HW timing."""
    EXT = FC + 2 * Cd
    nchunk = Fd // FC
    assert Fd % FC == 0

    from concourse.bacc import Bacc
    nc = Bacc()
    x_d = nc.declare_dram_parameter("x", [H, Fd], F32, isOutput=False)
    wmix_d = nc.declare_dram_parameter("wmix", [128, 128], F32, isOutput=False)
    band_d = nc.declare_dram_parameter("band", [3, 128, 128], F32, isOutput=False)
    y_d = nc.declare_dram_parameter("y", [H, Fd], F32, isOutput=True)
    cc_in = nc.dram_tensor("cc_in", [2], F32)
    if n_cores > 1:
        cc_out = nc.dram_tensor("cc_out", [2], F32, addr_space="Shared")

    with tile.TileContext(nc) as tc, ExitStack() as ctx:
        consts = ctx.enter_context(tc.tile_pool(name="consts", bufs=1))
        work = ctx.enter_context(tc.tile_pool(name="work", bufs=2))
        gpool = ctx.enter_context(tc.tile_pool(name="gpool", bufs=2))
        hpool = ctx.enter_context(tc.tile_pool(name="hpool", bufs=2))
        psip = ctx.enter_context(tc.tile_pool(name="psip", bufs=8))
        vkpool = ctx.enter_context(tc.tile_pool(name="vkpool", bufs=8))
        vpool = ctx.enter_context(tc.tile_pool(name="vpool", bufs=2))
        ypool = ctx.enter_context(tc.tile_pool(name="ypool", bufs=2))
        mixps = ctx.enter_context(tc.tile_pool(name="mixps", bufs=2, space="PSUM"))
        convps = ctx.enter_context(tc.tile_pool(name="convps", bufs=2, space="PSUM"))

        # ---- constants / inputs resident in SBUF ----
        xpad = consts.tile([128, Fd + 2 * Cd], F32)
        nc.gpsimd.memset(xpad[:, 0:Cd], 0.0)
        nc.gpsimd.memset(xpad[:, Cd + Fd:], 0.0)
        nc.sync.dma_start(out=xpad[:, Cd:Cd + Fd], in_=x_d[:, :])
        wmix_f = consts.tile([128, 128], F32)
        nc.sync.dma_start(out=wmix_f, in_=wmix_d[:, :])
        wmix = consts.tile([128, 128], BF16)
        nc.vector.tensor_copy(wmix, wmix_f)
        band_f = consts.tile([128, 3, 128], F32)
        nc.sync.dma_start(out=band_f, in_=band_d[:, :, :].rearrange("d i o -> i d o"))
        band = consts.tile([128, 3, 128], BF16)
        nc.vector.tensor_copy(band, band_f)

        # ---- global min/max -> scale (op-order identical to the reference) ----
        mn = consts.tile([128, 1], F32)
        mx = consts.tile([128, 1], F32)
        nc.vector.tensor_reduce(mn, xpad[:, Cd:Cd + Fd], axis=AX.X, op=OP.min)
        nc.vector.tensor_reduce(mx, xpad[:, Cd:Cd + Fd], axis=AX.X, op=OP.max)
        tmp = consts.tile([128, 2], F32)
        nc.vector.tensor_scalar_mul(tmp[:, 0:1], mn, -1.0)   # (-min, max)
        nc.vector.tensor_copy(tmp[:, 1:2], mx)
        red = consts.tile([1, 2], F32)
        nc.gpsimd.tensor_reduce(red, tmp, axis=AX.C, op=OP.max)
        pair = consts.tile([128, 2], F32)
        dma_in = nc.sync.dma_start(out=cc_in[:], in_=red)
        src = cc_in
        prev = dma_in
        if n_cores > 1:
            cc = nc.gpsimd.collective_compute(
                "AllReduce", OP.max,
                replica_groups=[list(range(n_cores))],
                ins=[cc_in.ap().opt()], outs=[cc_out.ap().opt()],
            )
            add_dep_helper(cc.ins, dma_in.ins, True, "cc waits dram write")
            src = cc_out
            prev = cc
        bcast = bass.AP(tensor=src.ap().tensor, offset=0, ap=[[0, 128], [1, 2]])
        dma_back = nc.sync.dma_start(out=pair[:, :], in_=bcast)
        add_dep_helper(dma_back.ins, prev.ins, True, "bcast waits dram ready")
        negxmin = pair[:, 0:1]
        gmax = pair[:, 1:2]

        rng = consts.tile([128, 1], F32)
        nc.vector.tensor_tensor(rng, gmax, negxmin, op=OP.add)   # xmax - xmin
        dd = consts.tile([128, 1], F32)
        nc.vector.tensor_scalar_add(dd, rng, float(np.float32(1e-8)))
        recip = consts.tile([128, 1], F32)
        nc.vector.reciprocal(recip, dd)
        scale = consts.tile([128, 1], F32)
        nc.vector.tensor_scalar_mul(scale, recip, 8.0)

        # ---- main streamed pipeline over free-dim chunks ----
        # Software-pipelined: conv/select of chunk c-1 is emitted after the
        # mix phase of chunk c, so the PE never waits on the unpack DMAs.
        def mix_phase(ci):
            cs = ci * FC  # xpad col cs .. cs+EXT covers data cols [cs-Cd, cs+FC+Cd)
            u = work.tile([128, EXT], F32, tag="u")
            nc.vector.tensor_scalar(u, xpad[:, cs:cs + EXT], negxmin, scale,
                                    op0=OP.add, op1=OP.mult)
            # g_j = (u >= j) as bf16 0/1 (mask + staircase factor).  DVE, not
            # GPSIMD: a [128,1152] gpsimd tensor_scalar measures ~19us on HW
            # (12x the cost-model estimate) — gpsimd is useless for this.
            g = gpool.tile([128, 7, EXT], BF16, tag="g")
            for j in range(1, 8):
                nc.vector.tensor_scalar(g[:, j - 1, :], u, float(j), None,
                                        op0=OP.is_ge)
            # basis planes: h_all[:,0,:] = bf16(x) via ACT, h_j = x_bf*g_j (exact)
            h_all = hpool.tile([128, 8, EXT], BF16, tag="h")
            nc.scalar.activation(h_all[:, 0, :], xpad[:, cs:cs + EXT], ACTF.Copy)
            for j in range(1, 8):
                nc.vector.tensor_tensor(h_all[:, j, :], h_all[:, 0, :],
                                        g[:, j - 1, :], op=OP.mult)

            # per 16-row h-group: pack -> mix matmul -> evacuate -> unpack.
            # groups are stride-8 partition combs (k, k+8, ..., k+120): a comb
            # spans 8 SDMA ports vs 4 for a contiguous 16-row block, doubling
            # per-DMA bandwidth (HWDGE queues drain DMAs serially).  packs and
            # unpacks alternate between the SP and ACT HWDGE queues so the two
            # queues run in parallel; an unpack (waiting on its PSUM
            # evacuation) must never sit ahead of packs on the same queue.
            v_all = vpool.tile([128, 8, EXT], BF16, tag="v")
            spans = [(s, min(EXT, s + 512)) for s in range(0, EXT, 512)]
            psis = []
            for grp in range(8):
                psi = psip.tile([128, EXT], BF16, tag="psi")
                q = nc.sync if grp % 2 == 0 else nc.scalar
                q.dma_start(out=psi, in_=h_all[grp::8, :, :])
                psis.append(psi)
            for grp in range(8):
                v_pk = vkpool.tile([128, EXT], BF16, tag="vpk")
                pm = mixps.tile([128, EXT], F32, tag="pm")
                for s, e in spans:
                    nc.tensor.matmul(pm[:, s:e], wmix, psis[grp][:, s:e],
                                     start=True, stop=True)
                # evacuate PSUM -> bf16 on ACT in one wide op (otherwise idle)
                nc.scalar.activation(v_pk, pm, ACTF.Copy)
                q = nc.scalar if grp % 2 == 0 else nc.sync
                q.dma_start(out=v_all[grp::8, :, :], in_=v_pk)
            return v_all, g

        def conv_phase(ci, v_all, g):
            cs = ci * FC
            # conv + select per 512-col output sub-chunk
            y_t = ypool.tile([128, FC], F32, tag="y")
            for sub in range(0, FC, 512):
                sw = min(512, FC - sub)
                for i in range(NB):
                    vt = convps.tile([128, sw], F32, tag="vt")
                    for dw in range(3):
                        nc.tensor.matmul(
                            vt, band[:, dw, :],
                            v_all[:, i, sub + dw * Cd:sub + dw * Cd + sw],
                            start=(dw == 0), stop=(dw == 2))
                    if i == 0:
                        nc.scalar.activation(y_t[:, sub:sub + sw], vt, ACTF.Copy)
                    else:
                        # overwrite where g_i != 0 (ascending i => y = V_idx)
                        mask = g[:, i - 1, Cd + sub:Cd + sub + sw].bitcast(
                            mybir.dt.uint16)
                        nc.vector.copy_predicated(y_t[:, sub:sub + sw], mask, vt)
            nc.scalar.dma_start(out=y_d[:, cs:cs + FC], in_=y_t)

        def pipeline():
            prev = None
            for ci in range(nchunk):
                cur = mix_phase(ci)
                if prev is not None:
                    conv_phase(ci - 1, *prev)
                prev = cur
            conv_phase(nchunk - 1, *prev)

        if reps == 1:
            pipeline()
        else:
            with tc.For_i(0, reps, 1):
                pipeline()
    nc.finalize()
    return nc


_CACHE = {}


def _run(x, co_matrix, w_spatial, trace=False):
    x = np.ascontiguousarray(np.asarray(x, np.float32))
    wmix, band = build_tables(co_matrix, w_spatial)
    n, h, w_, c = x.shape
    assert (n, h, w_, c) == (N, H, W, C), (n, h, w_, c)

    from concourse.bass_utils import run_bass_kernel_spmd

    key = "full"
    if key not in _CACHE:
        _CACHE[key] = build_bass(n_cores=N_CORES, FC=1024)
    nc = _CACHE[key]

    in_maps = []
    for core in range(N_CORES):
        in_maps.append({
            "x": x[core].reshape(H, W * C),
            "wmix": wmix,
            "band": band,
        })
    res = run_bass_kernel_spmd(nc, in_maps, core_ids=list(range(N_CORES)),
                               trace=trace)
    out = np.stack([res.results[i]["y"].reshape(H, W, C) for i in range(N_CORES)], 0)
    return out, res


def kernel(x, co_matrix, w_spatial):
    return _run(x, co_matrix, w_spatial)[0]


def run_traced(x, co_matrix, w_spatial):
    _, res = _run(x, co_matrix, w_spatial, trace=True)
    return res.exec_time_ns


# revision 11
# speedup vs baseline: 5.8259x; 1.3107x over previous
"""Trainium2 Bass kernel for the CoLL co-occurrence layer.

Math (per image):
    scale = 8/(max(x)-min(x)+1e-8)   (global over the whole batch)
    u     = (x - xmin)*scale ;  idx = clip(floor(u), 0, 7)
    y(p)  = sum_q w[q] * x(p+q) * co[idx_p, idx_q]   over 3x3 neighborhoods q

Reformulation (staircase basis, bf16 matmul pipeline):
    g_j(p) = 1[u(p) >= j]                       j = 1..7
    basis planes: h_0 = x, h_j = x*g_j          (bf16; h_j exact product since g is 0/1)
    v_i = x * co[i, idx] = sum_j A[i,j] h_j     with A[i,j] = co[i,j]-co[i,j-1]
    V_i = conv3x3(v_i, w)                       (SAME, zero pad)
    y(p) = V_{idx_p}(p)                         via chain of predicated copies on g_i

Mapping (one image per NeuronCore, [h=128 partitions, (w,c)=8192 free]):
  - global min/max via a 2-float AllReduce(max) of (-min, max).
  - the 8x8 A-mix runs on the PE with the packed layout partition=(8r+j) over
    16-row h-groups: one DMA packs h_all[16g:16g+16,:,:] -> psi[128,:], one
    matmul with stationary kron(I16, A^T) produces v packed as (8r+i), one DMA
    unpacks back to natural v planes.  x rides as basis plane 0, so the mix
    emits v_i = x*rho_i directly (no separate multiply, no bias).
  - conv along h via tridiagonal band matmuls (contraction over h_in), conv
    along w via +-C free-dim offsets of the zero-padded, EXT-wide v tiles.
  - all matmul operands bf16 (4x PE throughput vs f32); PSUM accumulates f32.
  - engine split: u + h_j + select on DVE, g_j on GPSIMD, x->bf16 and PSUM
    evacuation on ACT, pack/unpack/y DMAs on the SP (sync) HWDGE queue.
"""

from contextlib import ExitStack

import numpy as np

import concourse.bass as bass
import concourse.tile as tile
from concourse import mybir
from concourse.tile_rust import add_dep_helper

F32 = mybir.dt.float32
BF16 = mybir.dt.bfloat16
AX = mybir.AxisListType
OP = mybir.AluOpType
ACTF = mybir.ActivationFunctionType

N, H, W, C = 8, 128, 128, 64
NB = 8
N_CORES = 8
Fd = W * C
Cd = C


def build_tables(co, w):
    """Host-side weight construction (f32; converted to bf16 on-chip).

    wmix = kron(I16, A^T): stationary for the packed mix matmul.  With moving
    operand psi[8r+j, col] = h_j[16g+r, col] it yields out[8r+i] =
    sum_j A[i,j] h_j = v_i (packed).
    band[dw][hi, ho] = w[dh, dw] at hi = ho+dh-1: tridiagonal h-conv per
    w-offset, accumulated over dw in PSUM.
    """
    co = np.asarray(co, np.float32)
    w = np.asarray(w, np.float32)
    A = co - np.concatenate([np.zeros((NB, 1), np.float32), co[:, :-1]], axis=1)
    wmix = np.kron(np.eye(16, dtype=np.float32), A.T)          # [128, 128]
    band = np.zeros((3, 128, 128), np.float32)  # [dw, h_in, h_out]
    for dw in range(3):
        for ho in range(128):
            for dh in range(3):
                hi = ho + dh - 1
                if 0 <= hi < 128:
                    band[dw, hi, ho] = w[dh, dw]
    return wmix, band


def build_bass(n_cores=N_CORES, FC=1024, reps=1):
    """Per-core Bass module; same program on every core, collective min/max
    when n_cores > 1. reps>1 wraps the main pipeline in a For_i for wall-clock
    HW timing."""
    EXT = FC + 2 * Cd
    nchunk = Fd // FC
    assert Fd % FC == 0

    from concourse.bacc import Bacc
    nc = Bacc()
    x_d = nc.declare_dram_parameter("x", [H, Fd], F32, isOutput=False)
    wmix_d = nc.declare_dram_parameter("wmix", [128, 128], F32, isOutput=False)
    band_d = nc.declare_dram_parameter("band", [3, 128, 128], F32, isOutput=False)
    y_d = nc.declare_dram_parameter("y", [H, Fd], F32, isOutput=True)
    cc_in = nc.dram_tensor("cc_in", [2], F32)
    if n_cores > 1:
        cc_out = nc.dram_tensor("cc_out", [2], F32, addr_space="Shared")

    with tile.TileContext(nc) as tc, ExitStack() as ctx:
        consts = ctx.enter_context(tc.tile_pool(name="consts", bufs=1))
        work = ctx.enter_context(tc.tile_pool(name="work", bufs=2))
        hpool = ctx.enter_context(tc.tile_pool(name="hpool", bufs=2))
        psip = ctx.enter_context(tc.tile_pool(name="psip", bufs=8))
        vkpool = ctx.enter_context(tc.tile_pool(name="vkpool", bufs=8))
        vpool = ctx.enter_context(tc.tile_pool(name="vpool", bufs=2))
        ypool = ctx.enter_context(tc.tile_pool(name="ypool", bufs=2))
        mixps = ctx.enter_context(tc.tile_pool(name="mixps", bufs=2, space="PSUM"))
        convps = ctx.enter_context(tc.tile_pool(name="convps", bufs=2, space="PSUM"))

        # ---- constants / inputs resident in SBUF ----
        xpad = consts.tile([128, Fd + 2 * Cd], F32)
        nc.gpsimd.memset(xpad[:, 0:Cd], 0.0)
        nc.gpsimd.memset(xpad[:, Cd + Fd:], 0.0)
        nc.sync.dma_start(out=xpad[:, Cd:Cd + Fd], in_=x_d[:, :])
        wmix_f = consts.tile([128, 128], F32)
        nc.sync.dma_start(out=wmix_f, in_=wmix_d[:, :])
        wmix = consts.tile([128, 128], BF16)
        nc.vector.tensor_copy(wmix, wmix_f)
        band_f = consts.tile([128, 3, 128], F32)
        nc.sync.dma_start(out=band_f, in_=band_d[:, :, :].rearrange("d i o -> i d o"))
        band = consts.tile([128, 3, 128], BF16)
        nc.vector.tensor_copy(band, band_f)

        # ---- global min/max -> scale (op-order identical to the reference) ----
        mn = consts.tile([128, 1], F32)
        mx = consts.tile([128, 1], F32)
        nc.vector.tensor_reduce(mn, xpad[:, Cd:Cd + Fd], axis=AX.X, op=OP.min)
        nc.vector.tensor_reduce(mx, xpad[:, Cd:Cd + Fd], axis=AX.X, op=OP.max)
        tmp = consts.tile([128, 2], F32)
        nc.vector.tensor_scalar_mul(tmp[:, 0:1], mn, -1.0)   # (-min, max)
        nc.vector.tensor_copy(tmp[:, 1:2], mx)
        red = consts.tile([1, 2], F32)
        nc.gpsimd.tensor_reduce(red, tmp, axis=AX.C, op=OP.max)
        pair = consts.tile([128, 2], F32)
        dma_in = nc.sync.dma_start(out=cc_in[:], in_=red)
        src = cc_in
        prev = dma_in
        if n_cores > 1:
            cc = nc.gpsimd.collective_compute(
                "AllReduce", OP.max,
                replica_groups=[list(range(n_cores))],
                ins=[cc_in.ap().opt()], outs=[cc_out.ap().opt()],
            )
            add_dep_helper(cc.ins, dma_in.ins, True, "cc waits dram write")
            src = cc_out
            prev = cc
        bcast = bass.AP(tensor=src.ap().tensor, offset=0, ap=[[0, 128], [1, 2]])
        dma_back = nc.sync.dma_start(out=pair[:, :], in_=bcast)
        add_dep_helper(dma_back.ins, prev.ins, True, "bcast waits dram ready")
        negxmin = pair[:, 0:1]
        gmax = pair[:, 1:2]

        rng = consts.tile([128, 1], F32)
        nc.vector.tensor_tensor(rng, gmax, negxmin, op=OP.add)   # xmax - xmin
        dd = consts.tile([128, 1], F32)
        nc.vector.tensor_scalar_add(dd, rng, float(np.float32(1e-8)))
        recip = consts.tile([128, 1], F32)
        nc.vector.reciprocal(recip, dd)
        scale = consts.tile([128, 1], F32)
        nc.vector.tensor_scalar_mul(scale, recip, 8.0)

        # ---- main streamed pipeline over free-dim chunks ----
        # Software-pipelined: conv/select of chunk c-1 is emitted after the
        # mix phase of chunk c, so the PE never waits on the unpack DMAs.
        def mix_phase(ci):
            cs = ci * FC  # xpad col cs .. cs+EXT covers data cols [cs-Cd, cs+FC+Cd)
            u = work.tile([128, EXT], F32, tag="u")
            nc.vector.tensor_scalar(u, xpad[:, cs:cs + EXT], negxmin, scale,
                                    op0=OP.add, op1=OP.mult)
            # basis planes: h_all[:,0,:] = bf16(x) via ACT,
            # h_j = (u >= j) * x_bf in ONE fused DVE op (scalar_tensor_tensor).
            # h_j also serves as the select mask for bin j: h_j != 0 iff
            # g_j = 1 (u >= j >= 1 implies x > xmin implies x_bf > 0).
            # Not GPSIMD: a [128,1152] gpsimd tensor_scalar measures ~19us on
            # HW (12x the cost-model estimate) — gpsimd is useless here.
            h_all = hpool.tile([128, 8, EXT], BF16, tag="h")
            nc.scalar.activation(h_all[:, 0, :], xpad[:, cs:cs + EXT], ACTF.Copy)
            for j in range(1, 8):
                nc.vector.scalar_tensor_tensor(h_all[:, j, :], u, float(j),
                                               h_all[:, 0, :],
                                               op0=OP.is_ge, op1=OP.mult)

            # per 16-row h-group: pack -> mix matmul -> evacuate -> unpack.
            # groups are stride-8 partition combs (k, k+8, ..., k+120): a comb
            # spans 8 SDMA ports vs 4 for a contiguous 16-row block, doubling
            # per-DMA bandwidth (HWDGE queues drain DMAs serially).  packs and
            # unpacks alternate between the SP and ACT HWDGE queues so the two
            # queues run in parallel; an unpack (waiting on its PSUM
            # evacuation) must never sit ahead of packs on the same queue.
            v_all = vpool.tile([128, 8, EXT], BF16, tag="v")
            spans = [(s, min(EXT, s + 512)) for s in range(0, EXT, 512)]
            psis = []
            for grp in range(8):
                psi = psip.tile([128, EXT], BF16, tag="psi")
                q = nc.sync if grp % 2 == 0 else nc.scalar
                q.dma_start(out=psi, in_=h_all[grp::8, :, :])
                psis.append(psi)
            for grp in range(8):
                v_pk = vkpool.tile([128, EXT], BF16, tag="vpk")
                pm = mixps.tile([128, EXT], F32, tag="pm")
                for s, e in spans:
                    nc.tensor.matmul(pm[:, s:e], wmix, psis[grp][:, s:e],
                                     start=True, stop=True)
                # evacuate PSUM -> bf16 on ACT in one wide op (otherwise idle)
                nc.scalar.activation(v_pk, pm, ACTF.Copy)
                q = nc.scalar if grp % 2 == 0 else nc.sync
                q.dma_start(out=v_all[grp::8, :, :], in_=v_pk)
            return v_all, h_all

        def conv_phase(ci, v_all, h_all):
            cs = ci * FC
            # conv + select per 512-col output sub-chunk
            y_t = ypool.tile([128, FC], F32, tag="y")
            for sub in range(0, FC, 512):
                sw = min(512, FC - sub)
                for i in range(NB):
                    vt = convps.tile([128, sw], F32, tag="vt")
                    for dw in range(3):
                        nc.tensor.matmul(
                            vt, band[:, dw, :],
                            v_all[:, i, sub + dw * Cd:sub + dw * Cd + sw],
                            start=(dw == 0), stop=(dw == 2))
                    if i == 0:
                        nc.scalar.activation(y_t[:, sub:sub + sw], vt, ACTF.Copy)
                    else:
                        # overwrite where h_i != 0 (ascending i => y = V_idx)
                        mask = h_all[:, i, Cd + sub:Cd + sub + sw].bitcast(
                            mybir.dt.uint16)
                        nc.vector.copy_predicated(y_t[:, sub:sub + sw], mask, vt)
            nc.scalar.dma_start(out=y_d[:, cs:cs + FC], in_=y_t)

        def pipeline():
            prev = None
            for ci in range(nchunk):
                cur = mix_phase(ci)
                if prev is not None:
                    conv_phase(ci - 1, *prev)
                prev = cur
            conv_phase(nchunk - 1, *prev)

        if reps == 1:
            pipeline()
        else:
            with tc.For_i(0, reps, 1):
                pipeline()
    nc.finalize()
    return nc


_CACHE = {}


def _run(x, co_matrix, w_spatial, trace=False):
    x = np.ascontiguousarray(np.asarray(x, np.float32))
    wmix, band = build_tables(co_matrix, w_spatial)
    n, h, w_, c = x.shape
    assert (n, h, w_, c) == (N, H, W, C), (n, h, w_, c)

    from concourse.bass_utils import run_bass_kernel_spmd

    key = "full"
    if key not in _CACHE:
        _CACHE[key] = build_bass(n_cores=N_CORES, FC=1024)
    nc = _CACHE[key]

    in_maps = []
    for core in range(N_CORES):
        in_maps.append({
            "x": x[core].reshape(H, W * C),
            "wmix": wmix,
            "band": band,
        })
    res = run_bass_kernel_spmd(nc, in_maps, core_ids=list(range(N_CORES)),
                               trace=trace)
    out = np.stack([res.results[i]["y"].reshape(H, W, C) for i in range(N_CORES)], 0)
    return out, res


def kernel(x, co_matrix, w_spatial):
    return _run(x, co_matrix, w_spatial)[0]


def run_traced(x, co_matrix, w_spatial):
    _, res = _run(x, co_matrix, w_spatial, trace=True)
    return res.exec_time_ns


# revision 14
# speedup vs baseline: 6.0561x; 1.0395x over previous
"""Trainium2 Bass kernel for the CoLL co-occurrence layer.

Math (per image):
    scale = 8/(max(x)-min(x)+1e-8)   (global over the whole batch)
    u     = (x - xmin)*scale ;  idx = clip(floor(u), 0, 7)
    y(p)  = sum_q w[q] * x(p+q) * co[idx_p, idx_q]   over 3x3 neighborhoods q

Reformulation (staircase basis, bf16 matmul pipeline):
    g_j(p) = 1[u(p) >= j]                       j = 1..7
    basis planes: h_0 = x, h_j = x*g_j          (bf16; h_j exact product since g is 0/1)
    v_i = x * co[i, idx] = sum_j A[i,j] h_j     with A[i,j] = co[i,j]-co[i,j-1]
    V_i = conv3x3(v_i, w)                       (SAME, zero pad)
    y(p) = V_{idx_p}(p)                         via chain of predicated copies on g_i

Mapping (one image per NeuronCore, [h=128 partitions, (w,c)=8192 free]):
  - global min/max via a 2-float AllReduce(max) of (-min, max).
  - the 8x8 A-mix runs on the PE with the packed layout partition=(8r+j) over
    16-row h-groups: one DMA packs h_all[16g:16g+16,:,:] -> psi[128,:], one
    matmul with stationary kron(I16, A^T) produces v packed as (8r+i), one DMA
    unpacks back to natural v planes.  x rides as basis plane 0, so the mix
    emits v_i = x*rho_i directly (no separate multiply, no bias).
  - conv along h via tridiagonal band matmuls (contraction over h_in), conv
    along w via +-C free-dim offsets of the zero-padded, EXT-wide v tiles.
  - all matmul operands bf16 (4x PE throughput vs f32); PSUM accumulates f32.
  - engine split: u + h_j + select on DVE, g_j on GPSIMD, x->bf16 and PSUM
    evacuation on ACT, pack/unpack/y DMAs on the SP (sync) HWDGE queue.
"""

from contextlib import ExitStack

import numpy as np

import concourse.bass as bass
import concourse.tile as tile
from concourse import mybir
from concourse.tile_rust import add_dep_helper

F32 = mybir.dt.float32
BF16 = mybir.dt.bfloat16
AX = mybir.AxisListType
OP = mybir.AluOpType
ACTF = mybir.ActivationFunctionType

N, H, W, C = 8, 128, 128, 64
NB = 8
N_CORES = 8
Fd = W * C
Cd = C


def build_tables(co, w):
    """Host-side weight construction (f32; converted to bf16 on-chip).

    wmix = kron(I16, A^T): stationary for the packed mix matmul.  With moving
    operand psi[8r+j, col] = h_j[16g+r, col] it yields out[8r+i] =
    sum_j A[i,j] h_j = v_i (packed).
    band[dw][hi, ho] = w[dh, dw] at hi = ho+dh-1: tridiagonal h-conv per
    w-offset, accumulated over dw in PSUM.
    """
    co = np.asarray(co, np.float32)
    w = np.asarray(w, np.float32)
    A = co - np.concatenate([np.zeros((NB, 1), np.float32), co[:, :-1]], axis=1)
    wmix = np.kron(np.eye(16, dtype=np.float32), A.T)          # [128, 128]
    band = np.zeros((3, 128, 128), np.float32)  # [dw, h_in, h_out]
    for dw in range(3):
        for ho in range(128):
            for dh in range(3):
                hi = ho + dh - 1
                if 0 <= hi < 128:
                    band[dw, hi, ho] = w[dh, dw]
    return wmix, band


def build_bass(n_cores=N_CORES, FC=1024, reps=1):
    """Per-core Bass module; same program on every core, collective min/max
    when n_cores > 1. reps>1 wraps the main pipeline in a For_i for wall-clock
    HW timing."""
    EXT = FC + 2 * Cd
    nchunk = Fd // FC
    assert Fd % FC == 0

    from concourse.bacc import Bacc
    nc = Bacc()
    x_d = nc.declare_dram_parameter("x", [H, Fd], F32, isOutput=False)
    wmix_d = nc.declare_dram_parameter("wmix", [128, 128], F32, isOutput=False)
    band_d = nc.declare_dram_parameter("band", [3, 128, 128], F32, isOutput=False)
    y_d = nc.declare_dram_parameter("y", [H, Fd], F32, isOutput=True)
    cc_in = nc.dram_tensor("cc_in", [2], F32)
    if n_cores > 1:
        cc_out = nc.dram_tensor("cc_out", [2], F32, addr_space="Shared")

    with tile.TileContext(nc) as tc, ExitStack() as ctx:
        consts = ctx.enter_context(tc.tile_pool(name="consts", bufs=1))
        work = ctx.enter_context(tc.tile_pool(name="work", bufs=2))
        hpool = ctx.enter_context(tc.tile_pool(name="hpool", bufs=3))
        psip = ctx.enter_context(tc.tile_pool(name="psip", bufs=8))
        vkpool = ctx.enter_context(tc.tile_pool(name="vkpool", bufs=6))
        vpool = ctx.enter_context(tc.tile_pool(name="vpool", bufs=3))
        ypool = ctx.enter_context(tc.tile_pool(name="ypool", bufs=2))
        mixps = ctx.enter_context(tc.tile_pool(name="mixps", bufs=2, space="PSUM"))
        convps = ctx.enter_context(tc.tile_pool(name="convps", bufs=2, space="PSUM"))

        # ---- constants / inputs resident in SBUF ----
        xpad = consts.tile([128, Fd + 2 * Cd], F32)
        nc.gpsimd.memset(xpad[:, 0:Cd], 0.0)
        nc.gpsimd.memset(xpad[:, Cd + Fd:], 0.0)
        nc.sync.dma_start(out=xpad[:, Cd:Cd + Fd], in_=x_d[:, :])
        wmix_f = consts.tile([128, 128], F32)
        nc.sync.dma_start(out=wmix_f, in_=wmix_d[:, :])
        wmix = consts.tile([128, 128], BF16)
        nc.vector.tensor_copy(wmix, wmix_f)
        band_f = consts.tile([128, 3, 128], F32)
        nc.sync.dma_start(out=band_f, in_=band_d[:, :, :].rearrange("d i o -> i d o"))
        band = consts.tile([128, 3, 128], BF16)
        nc.vector.tensor_copy(band, band_f)

        # ---- global min/max -> scale (op-order identical to the reference) ----
        mn = consts.tile([128, 1], F32)
        mx = consts.tile([128, 1], F32)
        nc.vector.tensor_reduce(mn, xpad[:, Cd:Cd + Fd], axis=AX.X, op=OP.min)
        nc.vector.tensor_reduce(mx, xpad[:, Cd:Cd + Fd], axis=AX.X, op=OP.max)
        tmp = consts.tile([128, 2], F32)
        nc.vector.tensor_scalar_mul(tmp[:, 0:1], mn, -1.0)   # (-min, max)
        nc.vector.tensor_copy(tmp[:, 1:2], mx)
        red = consts.tile([1, 2], F32)
        nc.gpsimd.tensor_reduce(red, tmp, axis=AX.C, op=OP.max)
        pair = consts.tile([128, 2], F32)
        dma_in = nc.sync.dma_start(out=cc_in[:], in_=red)
        src = cc_in
        prev = dma_in
        if n_cores > 1:
            cc = nc.gpsimd.collective_compute(
                "AllReduce", OP.max,
                replica_groups=[list(range(n_cores))],
                ins=[cc_in.ap().opt()], outs=[cc_out.ap().opt()],
            )
            add_dep_helper(cc.ins, dma_in.ins, True, "cc waits dram write")
            src = cc_out
            prev = cc
        bcast = bass.AP(tensor=src.ap().tensor, offset=0, ap=[[0, 128], [1, 2]])
        dma_back = nc.sync.dma_start(out=pair[:, :], in_=bcast)
        add_dep_helper(dma_back.ins, prev.ins, True, "bcast waits dram ready")
        negxmin = pair[:, 0:1]
        gmax = pair[:, 1:2]

        rng = consts.tile([128, 1], F32)
        nc.vector.tensor_tensor(rng, gmax, negxmin, op=OP.add)   # xmax - xmin
        dd = consts.tile([128, 1], F32)
        nc.vector.tensor_scalar_add(dd, rng, float(np.float32(1e-8)))
        recip = consts.tile([128, 1], F32)
        nc.vector.reciprocal(recip, dd)
        scale = consts.tile([128, 1], F32)
        nc.vector.tensor_scalar_mul(scale, recip, 8.0)

        # ---- main streamed pipeline over free-dim chunks ----
        # Three-phase, lag-2 software pipeline per iteration:
        #   pre(c):    u, basis planes, pack DMAs        (DVE/ACT + queues)
        #   conv(c-2): conv matmuls + select + y         (PE/DVE)
        #   mix(c):    mix matmuls, evacuate, unpack     (PE/ACT + queues)
        # conv(c-2) sits between pre(c) and mix(c) so the in-order PE engine
        # runs (long-ready) conv work while the pack DMAs of chunk c drain,
        # and the chain depth per chunk is spread over ~3 chunks in flight.
        def pre_phase(ci):
            cs = ci * FC  # xpad col cs .. cs+EXT covers data cols [cs-Cd, cs+FC+Cd)
            u = work.tile([128, EXT], F32, tag="u")
            nc.vector.tensor_scalar(u, xpad[:, cs:cs + EXT], negxmin, scale,
                                    op0=OP.add, op1=OP.mult)
            # basis planes: h_all[:,0,:] = bf16(x) via ACT,
            # h_j = (u >= j) * x_bf in ONE fused DVE op (scalar_tensor_tensor).
            # h_j also serves as the select mask for bin j: h_j != 0 iff
            # g_j = 1 (u >= j >= 1 implies x > xmin implies x_bf > 0).
            # Not GPSIMD: a [128,1152] gpsimd tensor_scalar measures ~19us on
            # HW (12x the cost-model estimate) — gpsimd is useless here.
            h_all = hpool.tile([128, 8, EXT], BF16, tag="h")
            nc.scalar.activation(h_all[:, 0, :], xpad[:, cs:cs + EXT], ACTF.Copy)
            for j in range(1, 8):
                nc.vector.scalar_tensor_tensor(h_all[:, j, :], u, float(j),
                                               h_all[:, 0, :],
                                               op0=OP.is_ge, op1=OP.mult)
            # pack: groups are stride-8 partition combs (k, k+8, ..., k+120):
            # a comb spans 8 SDMA ports vs 4 for a contiguous 16-row block,
            # doubling per-DMA bandwidth (HWDGE queues drain DMAs serially).
            # packs and unpacks alternate between the SP and ACT HWDGE queues
            # so the two queues run in parallel; an unpack (waiting on its
            # PSUM evacuation) must never sit ahead of packs on its queue.
            psis = []
            for grp in range(8):
                psi = psip.tile([128, EXT], BF16, tag="psi")
                q = nc.sync if grp % 2 == 0 else nc.scalar
                q.dma_start(out=psi, in_=h_all[grp::8, :, :])
                psis.append(psi)
            return h_all, psis

        def mix_phase(ci, h_all, psis):
            v_all = vpool.tile([128, 8, EXT], BF16, tag="v")
            spans = [(s, min(EXT, s + 512)) for s in range(0, EXT, 512)]
            for grp in range(8):
                v_pk = vkpool.tile([128, EXT], BF16, tag="vpk")
                pm = mixps.tile([128, EXT], F32, tag="pm")
                for s, e in spans:
                    nc.tensor.matmul(pm[:, s:e], wmix, psis[grp][:, s:e],
                                     start=True, stop=True)
                # evacuate PSUM -> bf16 on ACT in one wide op (otherwise idle)
                nc.scalar.activation(v_pk, pm, ACTF.Copy)
                q = nc.scalar if grp % 2 == 0 else nc.sync
                q.dma_start(out=v_all[grp::8, :, :], in_=v_pk)
            return v_all, h_all

        def conv_phase(ci, v_all, h_all):
            cs = ci * FC
            # conv + select per 512-col output sub-chunk
            y_t = ypool.tile([128, FC], F32, tag="y")
            for sub in range(0, FC, 512):
                sw = min(512, FC - sub)
                for i in range(NB):
                    vt = convps.tile([128, sw], F32, tag="vt")
                    for dw in range(3):
                        nc.tensor.matmul(
                            vt, band[:, dw, :],
                            v_all[:, i, sub + dw * Cd:sub + dw * Cd + sw],
                            start=(dw == 0), stop=(dw == 2))
                    if i == 0:
                        nc.scalar.activation(y_t[:, sub:sub + sw], vt, ACTF.Copy)
                    else:
                        # overwrite where h_i != 0 (ascending i => y = V_idx)
                        mask = h_all[:, i, Cd + sub:Cd + sub + sw].bitcast(
                            mybir.dt.uint16)
                        nc.vector.copy_predicated(y_t[:, sub:sub + sw], mask, vt)
            nc.scalar.dma_start(out=y_d[:, cs:cs + FC], in_=y_t)

        def pipeline():
            ready = {}  # ci -> (v_all, h_all)
            for ci in range(nchunk):
                pre = pre_phase(ci)
                if ci - 2 in ready:
                    conv_phase(ci - 2, *ready.pop(ci - 2))
                ready[ci] = mix_phase(ci, *pre)
            for ci in (nchunk - 2, nchunk - 1):
                conv_phase(ci, *ready.pop(ci))

        if reps == 1:
            pipeline()
        else:
            with tc.For_i(0, reps, 1):
                pipeline()
    nc.finalize()
    return nc


_CACHE = {}


def _run(x, co_matrix, w_spatial, trace=False):
    x = np.ascontiguousarray(np.asarray(x, np.float32))
    wmix, band = build_tables(co_matrix, w_spatial)
    n, h, w_, c = x.shape
    assert (n, h, w_, c) == (N, H, W, C), (n, h, w_, c)

    from concourse.bass_utils import run_bass_kernel_spmd

    key = "full"
    if key not in _CACHE:
        _CACHE[key] = build_bass(n_cores=N_CORES, FC=1024)
    nc = _CACHE[key]

    in_maps = []
    for core in range(N_CORES):
        in_maps.append({
            "x": x[core].reshape(H, W * C),
            "wmix": wmix,
            "band": band,
        })
    res = run_bass_kernel_spmd(nc, in_maps, core_ids=list(range(N_CORES)),
                               trace=trace)
    out = np.stack([res.results[i]["y"].reshape(H, W, C) for i in range(N_CORES)], 0)
    return out, res


def kernel(x, co_matrix, w_spatial):
    return _run(x, co_matrix, w_spatial)[0]


def run_traced(x, co_matrix, w_spatial):
    _, res = _run(x, co_matrix, w_spatial, trace=True)
    return res.exec_time_ns


# revision 17
# speedup vs baseline: 6.3775x; 1.0531x over previous
"""Trainium2 Bass kernel for the CoLL co-occurrence layer.

Math (per image):
    scale = 8/(max(x)-min(x)+1e-8)   (global over the whole batch)
    u     = (x - xmin)*scale ;  idx = clip(floor(u), 0, 7)
    y(p)  = sum_q w[q] * x(p+q) * co[idx_p, idx_q]   over 3x3 neighborhoods q

Reformulation (staircase basis, bf16 matmul pipeline):
    g_j(p) = 1[u(p) >= j]                       j = 1..7
    basis planes: h_0 = x, h_j = x*g_j          (bf16; h_j exact product since g is 0/1)
    v_i = x * co[i, idx] = sum_j A[i,j] h_j     with A[i,j] = co[i,j]-co[i,j-1]
    V_i = conv3x3(v_i, w)                       (SAME, zero pad)
    y(p) = V_{idx_p}(p)                         via chain of predicated copies on g_i

Mapping (one image per NeuronCore, [h=128 partitions, (w,c)=8192 free]):
  - global min/max via a 2-float AllReduce(max) of (-min, max).
  - the 8x8 A-mix runs on the PE with the packed layout partition=(8r+j) over
    16-row h-groups: one DMA packs h_all[16g:16g+16,:,:] -> psi[128,:], one
    matmul with stationary kron(I16, A^T) produces v packed as (8r+i), one DMA
    unpacks back to natural v planes.  x rides as basis plane 0, so the mix
    emits v_i = x*rho_i directly (no separate multiply, no bias).
  - conv along h via tridiagonal band matmuls (contraction over h_in), conv
    along w via +-C free-dim offsets of the zero-padded, EXT-wide v tiles.
  - all matmul operands bf16 (4x PE throughput vs f32); PSUM accumulates f32.
  - engine split: u + h_j + select on DVE, g_j on GPSIMD, x->bf16 and PSUM
    evacuation on ACT, pack/unpack/y DMAs on the SP (sync) HWDGE queue.
"""

from contextlib import ExitStack

import numpy as np

import concourse.bass as bass
import concourse.tile as tile
from concourse import mybir
from concourse.tile_rust import add_dep_helper

F32 = mybir.dt.float32
BF16 = mybir.dt.bfloat16
AX = mybir.AxisListType
OP = mybir.AluOpType
ACTF = mybir.ActivationFunctionType

N, H, W, C = 8, 128, 128, 64
NB = 8
N_CORES = 8
Fd = W * C
Cd = C


def build_tables(co, w):
    """Host-side weight construction (f32; converted to bf16 on-chip).

    wmix = kron(I16, A^T): stationary for the packed mix matmul.  With moving
    operand psi[8r+j, col] = h_j[16g+r, col] it yields out[8r+i] =
    sum_j A[i,j] h_j = v_i (packed).
    band[dw][hi, ho] = w[dh, dw] at hi = ho+dh-1: tridiagonal h-conv per
    w-offset, accumulated over dw in PSUM.
    """
    co = np.asarray(co, np.float32)
    w = np.asarray(w, np.float32)
    A = co - np.concatenate([np.zeros((NB, 1), np.float32), co[:, :-1]], axis=1)
    wmix = np.kron(np.eye(16, dtype=np.float32), A.T)          # [128, 128]
    band = np.zeros((3, 128, 128), np.float32)  # [dw, h_in, h_out]
    for dw in range(3):
        for ho in range(128):
            for dh in range(3):
                hi = ho + dh - 1
                if 0 <= hi < 128:
                    band[dw, hi, ho] = w[dh, dw]
    return wmix, band


def build_bass(n_cores=N_CORES, FC=1024, reps=1, skip=()):
    """Per-core Bass module; same program on every core, collective min/max
    when n_cores > 1. reps>1 wraps the main pipeline in a For_i for wall-clock
    HW timing.  skip: benchmarking-only phase knockouts ("pre","pack","mix",
    "conv","sel") — output is garbage when used."""
    EXT = FC + 2 * Cd
    nchunk = Fd // FC
    assert Fd % FC == 0

    from concourse.bacc import Bacc
    nc = Bacc()
    x_d = nc.declare_dram_parameter("x", [H, Fd], F32, isOutput=False)
    wmix_d = nc.declare_dram_parameter("wmix", [128, 128], F32, isOutput=False)
    band_d = nc.declare_dram_parameter("band", [3, 128, 128], F32, isOutput=False)
    y_d = nc.declare_dram_parameter("y", [H, Fd], F32, isOutput=True)
    cc_in = nc.dram_tensor("cc_in", [2], F32)
    if n_cores > 1:
        cc_out = nc.dram_tensor("cc_out", [2], F32, addr_space="Shared")

    with tile.TileContext(nc) as tc, ExitStack() as ctx:
        consts = ctx.enter_context(tc.tile_pool(name="consts", bufs=1))
        work = ctx.enter_context(tc.tile_pool(name="work", bufs=2))
        hpool = ctx.enter_context(tc.tile_pool(name="hpool", bufs=3))
        psip = ctx.enter_context(tc.tile_pool(name="psip", bufs=8))
        vkpool = ctx.enter_context(tc.tile_pool(name="vkpool", bufs=6))
        vpool = ctx.enter_context(tc.tile_pool(name="vpool", bufs=3))
        ypool = ctx.enter_context(tc.tile_pool(name="ypool", bufs=2))
        mixps = ctx.enter_context(tc.tile_pool(name="mixps", bufs=2, space="PSUM"))
        convps = ctx.enter_context(tc.tile_pool(name="convps", bufs=2, space="PSUM"))

        # ---- constants / inputs resident in SBUF ----
        xpad = consts.tile([128, Fd + 2 * Cd], F32)
        nc.gpsimd.memset(xpad[:, 0:Cd], 0.0)
        nc.gpsimd.memset(xpad[:, Cd + Fd:], 0.0)
        nc.sync.dma_start(out=xpad[:, Cd:Cd + Fd], in_=x_d[:, :])
        wmix_f = consts.tile([128, 128], F32)
        nc.sync.dma_start(out=wmix_f, in_=wmix_d[:, :])
        wmix = consts.tile([128, 128], BF16)
        nc.vector.tensor_copy(wmix, wmix_f)
        band_f = consts.tile([128, 3, 128], F32)
        nc.sync.dma_start(out=band_f, in_=band_d[:, :, :].rearrange("d i o -> i d o"))
        band = consts.tile([128, 3, 128], BF16)
        nc.vector.tensor_copy(band, band_f)

        # ---- global min/max -> scale (op-order identical to the reference) ----
        mn = consts.tile([128, 1], F32)
        mx = consts.tile([128, 1], F32)
        nc.vector.tensor_reduce(mn, xpad[:, Cd:Cd + Fd], axis=AX.X, op=OP.min)
        nc.vector.tensor_reduce(mx, xpad[:, Cd:Cd + Fd], axis=AX.X, op=OP.max)
        tmp = consts.tile([128, 2], F32)
        nc.vector.tensor_scalar_mul(tmp[:, 0:1], mn, -1.0)   # (-min, max)
        nc.vector.tensor_copy(tmp[:, 1:2], mx)
        red = consts.tile([1, 2], F32)
        nc.gpsimd.tensor_reduce(red, tmp, axis=AX.C, op=OP.max)
        pair = consts.tile([128, 2], F32)
        dma_in = nc.sync.dma_start(out=cc_in[:], in_=red)
        src = cc_in
        prev = dma_in
        if n_cores > 1:
            cc = nc.gpsimd.collective_compute(
                "AllReduce", OP.max,
                replica_groups=[list(range(n_cores))],
                ins=[cc_in.ap().opt()], outs=[cc_out.ap().opt()],
            )
            add_dep_helper(cc.ins, dma_in.ins, True, "cc waits dram write")
            src = cc_out
            prev = cc
        bcast = bass.AP(tensor=src.ap().tensor, offset=0, ap=[[0, 128], [1, 2]])
        dma_back = nc.sync.dma_start(out=pair[:, :], in_=bcast)
        add_dep_helper(dma_back.ins, prev.ins, True, "bcast waits dram ready")
        negxmin = pair[:, 0:1]
        gmax = pair[:, 1:2]

        rng = consts.tile([128, 1], F32)
        nc.vector.tensor_tensor(rng, gmax, negxmin, op=OP.add)   # xmax - xmin
        dd = consts.tile([128, 1], F32)
        nc.vector.tensor_scalar_add(dd, rng, float(np.float32(1e-8)))
        recip = consts.tile([128, 1], F32)
        nc.vector.reciprocal(recip, dd)
        scale = consts.tile([128, 1], F32)
        nc.vector.tensor_scalar_mul(scale, recip, 8.0)

        # ---- main streamed pipeline over free-dim chunks ----
        # Three-phase, lag-2 software pipeline per iteration:
        #   pre(c):    u, basis planes, pack DMAs        (DVE/ACT + queues)
        #   conv(c-2): conv matmuls + select + y         (PE/DVE)
        #   mix(c):    mix matmuls, evacuate, unpack     (PE/ACT + queues)
        # conv(c-2) sits between pre(c) and mix(c) so the in-order PE engine
        # runs (long-ready) conv work while the pack DMAs of chunk c drain,
        # and the chain depth per chunk is spread over ~3 chunks in flight.
        def pre_phase(ci):
            cs = ci * FC  # xpad col cs .. cs+EXT covers data cols [cs-Cd, cs+FC+Cd)
            u = work.tile([128, EXT], F32, tag="u")
            nc.vector.tensor_scalar(u, xpad[:, cs:cs + EXT], negxmin, scale,
                                    op0=OP.add, op1=OP.mult)
            # basis planes: h_all[:,0,:] = bf16(x) via ACT,
            # h_j = (u >= j) * x_bf in ONE fused DVE op (scalar_tensor_tensor).
            # h_j also serves as the select mask for bin j: h_j != 0 iff
            # g_j = 1 (u >= j >= 1 implies x > xmin implies x_bf > 0).
            # Not GPSIMD: a [128,1152] gpsimd tensor_scalar measures ~19us on
            # HW (12x the cost-model estimate) — gpsimd is useless here.
            h_all = hpool.tile([128, 8, EXT], BF16, tag="h")
            if "pre" not in skip:
                nc.scalar.activation(h_all[:, 0, :], xpad[:, cs:cs + EXT],
                                     ACTF.Copy)
                for j in range(1, 8):
                    nc.vector.scalar_tensor_tensor(h_all[:, j, :], u, float(j),
                                                   h_all[:, 0, :],
                                                   op0=OP.is_ge, op1=OP.mult)
            else:
                nc.gpsimd.memset(h_all[:, 0:1, 0:4], 0.0)
            # pack: groups are stride-8 partition combs (k, k+8, ..., k+120):
            # a comb spans 8 SDMA ports vs 4 for a contiguous 16-row block,
            # doubling per-DMA bandwidth (HWDGE queues drain DMAs serially).
            # packs and unpacks alternate between the SP and ACT HWDGE queues
            # so the two queues run in parallel; an unpack (waiting on its
            # PSUM evacuation) must never sit ahead of packs on its queue.
            psis = []
            for grp in range(8):
                psi = psip.tile([128, EXT], BF16, tag="psi")
                if "pack" not in skip:
                    q = nc.sync if grp % 2 == 0 else nc.scalar
                    q.dma_start(out=psi, in_=h_all[grp::8, :, :])
                else:
                    nc.gpsimd.memset(psi[:, 0:4], 0.0)
                psis.append(psi)
            return h_all, psis

        def mix_phase(ci, h_all, psis):
            v_all = vpool.tile([128, 8, EXT], BF16, tag="v")
            spans = [(s, min(EXT, s + 512)) for s in range(0, EXT, 512)]
            if "mix" in skip:
                nc.gpsimd.memset(v_all[:, 0:1, 0:4], 0.0)
                return v_all, h_all
            for grp in range(8):
                v_pk = vkpool.tile([128, EXT], BF16, tag="vpk")
                pm = mixps.tile([128, EXT], F32, tag="pm")
                for s, e in spans:
                    nc.tensor.matmul(pm[:, s:e], wmix, psis[grp][:, s:e],
                                     start=True, stop=True)
                # evacuate PSUM -> bf16 on ACT in one wide op (otherwise idle)
                nc.scalar.activation(v_pk, pm, ACTF.Copy)
                q = nc.scalar if grp % 2 == 0 else nc.sync
                q.dma_start(out=v_all[grp::8, :, :], in_=v_pk)
            return v_all, h_all

        def conv_phase(ci, v_all, h_all):
            cs = ci * FC
            # conv + select per 512-col output sub-chunk
            y_t = ypool.tile([128, FC], F32, tag="y")
            if "conv" in skip:
                nc.gpsimd.memset(y_t[:, 0:4], 0.0)
                nc.scalar.dma_start(out=y_d[:, cs:cs + FC], in_=y_t)
                return
            for sub in range(0, FC, 512):
                sw = min(512, FC - sub)
                for i in range(NB):
                    vt = convps.tile([128, sw], F32, tag="vt")
                    for dw in range(3):
                        nc.tensor.matmul(
                            vt, band[:, dw, :],
                            v_all[:, i, sub + dw * Cd:sub + dw * Cd + sw],
                            start=(dw == 0), stop=(dw == 2))
                    if "sel" in skip:
                        continue
                    if i == 0:
                        nc.scalar.activation(y_t[:, sub:sub + sw], vt, ACTF.Copy)
                    else:
                        # overwrite where h_i != 0 (ascending i => y = V_idx)
                        mask = h_all[:, i, Cd + sub:Cd + sub + sw].bitcast(
                            mybir.dt.uint16)
                        nc.vector.copy_predicated(y_t[:, sub:sub + sw], mask, vt)
            if "sel" in skip:
                nc.gpsimd.memset(y_t[:, 0:4], 0.0)
            nc.scalar.dma_start(out=y_d[:, cs:cs + FC], in_=y_t)

        def pipeline(seq):
            ready = {}  # t -> (ci, v_all, h_all)
            for t, ci in enumerate(seq):
                pre = pre_phase(ci)
                if t - 2 in ready:
                    conv_phase(*ready.pop(t - 2))
                ready[t] = (ci,) + mix_phase(ci, *pre)
            for t in (len(seq) - 2, len(seq) - 1):
                conv_phase(*ready.pop(t))

        if reps == 1:
            pipeline(list(range(nchunk)))
        else:
            # For_i places an all-engine barrier in every iteration's
            # semaphore-reset block, so each iteration pays the full pipeline
            # fill+drain.  Unroll UNROLL pipeline copies per iteration (lag
            # flows continuously across copies) to amortize it.
            UNROLL = 4
            assert reps % UNROLL == 0, reps
            with tc.For_i(0, reps // UNROLL, 1):
                pipeline(list(range(nchunk)) * UNROLL)
    nc.finalize()
    return nc


_CACHE = {}


def _run(x, co_matrix, w_spatial, trace=False):
    x = np.ascontiguousarray(np.asarray(x, np.float32))
    wmix, band = build_tables(co_matrix, w_spatial)
    n, h, w_, c = x.shape
    assert (n, h, w_, c) == (N, H, W, C), (n, h, w_, c)

    from concourse.bass_utils import run_bass_kernel_spmd

    key = "full"
    if key not in _CACHE:
        _CACHE[key] = build_bass(n_cores=N_CORES, FC=1024)
    nc = _CACHE[key]

    in_maps = []
    for core in range(N_CORES):
        in_maps.append({
            "x": x[core].reshape(H, W * C),
            "wmix": wmix,
            "band": band,
        })
    res = run_bass_kernel_spmd(nc, in_maps, core_ids=list(range(N_CORES)),
                               trace=trace)
    out = np.stack([res.results[i]["y"].reshape(H, W, C) for i in range(N_CORES)], 0)
    return out, res


def kernel(x, co_matrix, w_spatial):
    return _run(x, co_matrix, w_spatial)[0]


def run_traced(x, co_matrix, w_spatial):
    _, res = _run(x, co_matrix, w_spatial, trace=True)
    return res.exec_time_ns


# revision 20
# speedup vs baseline: 9.3519x; 1.4664x over previous
"""Trainium2 Bass kernel for the CoLL co-occurrence layer.

Math (per image):
    scale = 8/(max(x)-min(x)+1e-8)   (global over the whole batch)
    u     = (x - xmin)*scale ;  idx = clip(floor(u), 0, 7)
    y(p)  = sum_q w[q] * x(p+q) * co[idx_p, idx_q]   over 3x3 neighborhoods q

Reformulation (staircase basis, bf16 matmul pipeline):
    g_j(p) = 1[u(p) >= j]                       j = 1..7
    basis planes: h_0 = x, h_j = x*g_j          (bf16; h_j exact product since g is 0/1)
    v_i = x * co[i, idx] = sum_j A[i,j] h_j     with A[i,j] = co[i,j]-co[i,j-1]
    V_i = conv3x3(v_i, w)                       (SAME, zero pad)
    y(p) = V_{idx_p}(p)                         via chain of predicated copies on g_i

Mapping (one image per NeuronCore, [h=128 partitions, (w,c)=8192 free]):
  - global min/max via a 2-float AllReduce(max) of (-min, max).
  - the 8x8 A-mix runs on the PE with the packed layout partition=(8r+j) over
    16-row h-groups: one DMA packs h_all[16g:16g+16,:,:] -> psi[128,:], one
    matmul with stationary kron(I16, A^T) produces v packed as (8r+i), one DMA
    unpacks back to natural v planes.  x rides as basis plane 0, so the mix
    emits v_i = x*rho_i directly (no separate multiply, no bias).
  - conv along h via tridiagonal band matmuls (contraction over h_in), conv
    along w via +-C free-dim offsets of the zero-padded, EXT-wide v tiles.
  - all matmul operands bf16 (4x PE throughput vs f32); PSUM accumulates f32.
  - engine split: u + h_j + select on DVE, g_j on GPSIMD, x->bf16 and PSUM
    evacuation on ACT, pack/unpack/y DMAs on the SP (sync) HWDGE queue.
"""

from contextlib import ExitStack

import numpy as np

import concourse.bass as bass
import concourse.tile as tile
from concourse import mybir
from concourse.tile_rust import add_dep_helper

F32 = mybir.dt.float32
BF16 = mybir.dt.bfloat16
AX = mybir.AxisListType
OP = mybir.AluOpType
ACTF = mybir.ActivationFunctionType

N, H, W, C = 8, 128, 128, 64
NB = 8
N_CORES = 8
Fd = W * C
Cd = C


def build_tables(co, w):
    """Host-side weight construction (f32; converted to bf16 on-chip).

    wmix = kron(I16, A^T): stationary for the packed mix matmul.  With moving
    operand psi[8r+j, col] = h_j[16g+r, col] it yields out[8r+i] =
    sum_j A[i,j] h_j = v_i (packed).
    band[dw][hi, ho] = w[dh, dw] at hi = ho+dh-1: tridiagonal h-conv per
    w-offset, accumulated over dw in PSUM.
    """
    co = np.asarray(co, np.float32)
    w = np.asarray(w, np.float32)
    A = co - np.concatenate([np.zeros((NB, 1), np.float32), co[:, :-1]], axis=1)
    wmix = np.kron(np.eye(16, dtype=np.float32), A.T)          # [128, 128]
    band = np.zeros((3, 128, 128), np.float32)  # [dw, h_in, h_out]
    for dw in range(3):
        for ho in range(128):
            for dh in range(3):
                hi = ho + dh - 1
                if 0 <= hi < 128:
                    band[dw, hi, ho] = w[dh, dw]
    return wmix, band


def build_bass(n_cores=N_CORES, FC=1024, reps=1, skip=()):
    """Per-core Bass module; same program on every core, collective min/max
    when n_cores > 1. reps>1 wraps the main pipeline in a For_i for wall-clock
    HW timing.  skip: benchmarking-only phase knockouts ("pre","pack","mix",
    "conv","sel") — output is garbage when used."""
    EXT = FC + 2 * Cd
    nchunk = Fd // FC
    assert Fd % FC == 0

    from concourse.bacc import Bacc
    nc = Bacc()
    x_d = nc.declare_dram_parameter("x", [H, Fd], F32, isOutput=False)
    wmix_d = nc.declare_dram_parameter("wmix", [128, 128], F32, isOutput=False)
    band_d = nc.declare_dram_parameter("band", [3, 128, 128], F32, isOutput=False)
    y_d = nc.declare_dram_parameter("y", [H, Fd], F32, isOutput=True)
    cc_in = nc.dram_tensor("cc_in", [2], F32)
    if n_cores > 1:
        cc_out = nc.dram_tensor("cc_out", [2], F32, addr_space="Shared")

    with tile.TileContext(nc) as tc, ExitStack() as ctx:
        consts = ctx.enter_context(tc.tile_pool(name="consts", bufs=1))
        work = ctx.enter_context(tc.tile_pool(name="work", bufs=2))
        hpool = ctx.enter_context(tc.tile_pool(name="hpool", bufs=3))
        psip = ctx.enter_context(tc.tile_pool(name="psip", bufs=8))
        vkpool = ctx.enter_context(tc.tile_pool(name="vkpool", bufs=6))
        vpool = ctx.enter_context(tc.tile_pool(name="vpool", bufs=3))
        ypool = ctx.enter_context(tc.tile_pool(name="ypool", bufs=2))
        mixps = ctx.enter_context(tc.tile_pool(name="mixps", bufs=2, space="PSUM"))
        convps = ctx.enter_context(tc.tile_pool(name="convps", bufs=2, space="PSUM"))

        # ---- constants / inputs resident in SBUF ----
        xpad = consts.tile([128, Fd + 2 * Cd], F32)
        nc.gpsimd.memset(xpad[:, 0:Cd], 0.0)
        nc.gpsimd.memset(xpad[:, Cd + Fd:], 0.0)
        nc.sync.dma_start(out=xpad[:, Cd:Cd + Fd], in_=x_d[:, :])
        wmix_f = consts.tile([128, 128], F32)
        nc.sync.dma_start(out=wmix_f, in_=wmix_d[:, :])
        wmix = consts.tile([128, 128], BF16)
        nc.vector.tensor_copy(wmix, wmix_f)
        band_f = consts.tile([128, 3, 128], F32)
        nc.sync.dma_start(out=band_f, in_=band_d[:, :, :].rearrange("d i o -> i d o"))
        band = consts.tile([128, 3, 128], BF16)
        nc.vector.tensor_copy(band, band_f)

        # ---- global min/max -> scale (op-order identical to the reference) ----
        mn = consts.tile([128, 1], F32)
        mx = consts.tile([128, 1], F32)
        nc.vector.tensor_reduce(mn, xpad[:, Cd:Cd + Fd], axis=AX.X, op=OP.min)
        nc.vector.tensor_reduce(mx, xpad[:, Cd:Cd + Fd], axis=AX.X, op=OP.max)
        tmp = consts.tile([128, 2], F32)
        nc.vector.tensor_scalar_mul(tmp[:, 0:1], mn, -1.0)   # (-min, max)
        nc.vector.tensor_copy(tmp[:, 1:2], mx)
        red = consts.tile([1, 2], F32)
        nc.gpsimd.tensor_reduce(red, tmp, axis=AX.C, op=OP.max)
        pair = consts.tile([128, 2], F32)
        dma_in = nc.sync.dma_start(out=cc_in[:], in_=red)
        src = cc_in
        prev = dma_in
        if n_cores > 1:
            cc = nc.gpsimd.collective_compute(
                "AllReduce", OP.max,
                replica_groups=[list(range(n_cores))],
                ins=[cc_in.ap().opt()], outs=[cc_out.ap().opt()],
            )
            add_dep_helper(cc.ins, dma_in.ins, True, "cc waits dram write")
            src = cc_out
            prev = cc
        bcast = bass.AP(tensor=src.ap().tensor, offset=0, ap=[[0, 128], [1, 2]])
        dma_back = nc.sync.dma_start(out=pair[:, :], in_=bcast)
        add_dep_helper(dma_back.ins, prev.ins, True, "bcast waits dram ready")
        negxmin = pair[:, 0:1]
        gmax = pair[:, 1:2]

        rng = consts.tile([128, 1], F32)
        nc.vector.tensor_tensor(rng, gmax, negxmin, op=OP.add)   # xmax - xmin
        dd = consts.tile([128, 1], F32)
        nc.vector.tensor_scalar_add(dd, rng, float(np.float32(1e-8)))
        recip = consts.tile([128, 1], F32)
        nc.vector.reciprocal(recip, dd)
        scale = consts.tile([128, 1], F32)
        nc.vector.tensor_scalar_mul(scale, recip, 8.0)

        if skip:
            zh = consts.tile([128, 8, EXT], BF16, name="zh")
            nc.vector.memset(zh, 0.0)
            zpsi = consts.tile([128, EXT], BF16, name="zpsi")
            nc.vector.memset(zpsi, 0.0)
            zy = consts.tile([128, FC], F32, name="zy")
            nc.vector.memset(zy, 0.0)

        # ---- main streamed pipeline over free-dim chunks ----
        # Three-phase, lag-2 software pipeline per iteration:
        #   pre(c):    u, basis planes, pack DMAs        (DVE/ACT + queues)
        #   conv(c-2): conv matmuls + select + y         (PE/DVE)
        #   mix(c):    mix matmuls, evacuate, unpack     (PE/ACT + queues)
        # conv(c-2) sits between pre(c) and mix(c) so the in-order PE engine
        # runs (long-ready) conv work while the pack DMAs of chunk c drain,
        # and the chain depth per chunk is spread over ~3 chunks in flight.
        def pre_phase(ci):
            cs = ci * FC  # xpad col cs .. cs+EXT covers data cols [cs-Cd, cs+FC+Cd)
            u = work.tile([128, EXT], F32, tag="u")
            nc.vector.tensor_scalar(u, xpad[:, cs:cs + EXT], negxmin, scale,
                                    op0=OP.add, op1=OP.mult)
            # basis planes: h_all[:,0,:] = bf16(x) via ACT,
            # h_j = (u >= j) * x_bf in ONE fused DVE op (scalar_tensor_tensor).
            # h_j also serves as the select mask for bin j: h_j != 0 iff
            # g_j = 1 (u >= j >= 1 implies x > xmin implies x_bf > 0).
            # Not GPSIMD: a [128,1152] gpsimd tensor_scalar measures ~19us on
            # HW (12x the cost-model estimate) — gpsimd is useless here.
            if "pre" not in skip:
                h_all = hpool.tile([128, 8, EXT], BF16, tag="h")
                nc.scalar.activation(h_all[:, 0, :], xpad[:, cs:cs + EXT],
                                     ACTF.Copy)
                for j in range(1, 8):
                    nc.vector.scalar_tensor_tensor(h_all[:, j, :], u, float(j),
                                                   h_all[:, 0, :],
                                                   op0=OP.is_ge, op1=OP.mult)
            else:
                h_all = zh
            # pack: groups are stride-8 partition combs (k, k+8, ..., k+120):
            # a comb spans 8 SDMA ports vs 4 for a contiguous 16-row block,
            # doubling per-DMA bandwidth (HWDGE queues drain DMAs serially).
            # packs and unpacks alternate between the SP and ACT HWDGE queues
            # so the two queues run in parallel; an unpack (waiting on its
            # PSUM evacuation) must never sit ahead of packs on its queue.
            # comb k touches SBUF-port set A for k<4, set B for k>=4: give SP
            # all set-A packs and ACT all set-B so the two queues' concurrent
            # transfers never collide on ports.
            psis = []
            for grp in range(8):
                if "pack" not in skip:
                    psi = psip.tile([128, EXT], BF16, tag="psi")
                    q = nc.sync if grp < 4 else nc.scalar
                    q.dma_start(out=psi, in_=h_all[grp::8, :, :])
                else:
                    psi = zpsi
                psis.append(psi)
            return h_all, psis

        def mix_phase(ci, h_all, psis):
            v_all = vpool.tile([128, 8, EXT], BF16, tag="v")
            spans = [(s, min(EXT, s + 512)) for s in range(0, EXT, 512)]
            if "mix" in skip:
                return zh, h_all
            # process groups A,B,A,B,... so unpack readiness alternates port
            # sets; set-A unpacks go to SP, set-B to ACT — concurrent drains
            # on the two queues then always use disjoint SBUF ports.
            for grp in (0, 4, 1, 5, 2, 6, 3, 7):
                v_pk = vkpool.tile([128, EXT], BF16, tag="vpk")
                pm = mixps.tile([128, EXT], F32, tag="pm")
                for s, e in spans:
                    nc.tensor.matmul(pm[:, s:e], wmix, psis[grp][:, s:e],
                                     start=True, stop=True)
                # evacuate PSUM -> bf16 on ACT in one wide op (otherwise idle)
                nc.scalar.activation(v_pk, pm, ACTF.Copy)
                q = nc.sync if grp < 4 else nc.scalar
                q.dma_start(out=v_all[grp::8, :, :], in_=v_pk)
            return v_all, h_all

        def conv_phase(ci, v_all, h_all):
            cs = ci * FC
            # conv + select per 512-col output sub-chunk
            if "conv" in skip:
                nc.scalar.dma_start(out=y_d[:, cs:cs + FC], in_=zy)
                return
            y_t = ypool.tile([128, FC], F32, tag="y")
            for sub in range(0, FC, 512):
                sw = min(512, FC - sub)
                for i in range(NB):
                    vt = convps.tile([128, sw], F32, tag="vt")
                    for dw in range(3):
                        nc.tensor.matmul(
                            vt, band[:, dw, :],
                            v_all[:, i, sub + dw * Cd:sub + dw * Cd + sw],
                            start=(dw == 0), stop=(dw == 2))
                    if "sel" in skip:
                        continue
                    if i == 0:
                        nc.scalar.activation(y_t[:, sub:sub + sw], vt, ACTF.Copy)
                    else:
                        # overwrite where h_i != 0 (ascending i => y = V_idx)
                        mask = h_all[:, i, Cd + sub:Cd + sub + sw].bitcast(
                            mybir.dt.uint16)
                        nc.vector.copy_predicated(y_t[:, sub:sub + sw], mask, vt)
            if "sel" in skip:
                nc.scalar.dma_start(out=y_d[:, cs:cs + FC], in_=zy)
            else:
                nc.scalar.dma_start(out=y_d[:, cs:cs + FC], in_=y_t)

        def pipeline(seq):
            ready = {}  # t -> (ci, v_all, h_all)
            for t, ci in enumerate(seq):
                pre = pre_phase(ci)
                if t - 2 in ready:
                    conv_phase(*ready.pop(t - 2))
                ready[t] = (ci,) + mix_phase(ci, *pre)
            for t in (len(seq) - 2, len(seq) - 1):
                conv_phase(*ready.pop(t))

        if reps == 1:
            pipeline(list(range(nchunk)))
        else:
            # For_i places an all-engine barrier in every iteration's
            # semaphore-reset block, so each iteration pays the full pipeline
            # fill+drain.  Unroll UNROLL pipeline copies per iteration (lag
            # flows continuously across copies) to amortize it.
            UNROLL = 4
            assert reps % UNROLL == 0, reps
            with tc.For_i(0, reps // UNROLL, 1):
                pipeline(list(range(nchunk)) * UNROLL)
    nc.finalize()
    return nc


_CACHE = {}


def _run(x, co_matrix, w_spatial, trace=False):
    x = np.ascontiguousarray(np.asarray(x, np.float32))
    wmix, band = build_tables(co_matrix, w_spatial)
    n, h, w_, c = x.shape
    assert (n, h, w_, c) == (N, H, W, C), (n, h, w_, c)

    from concourse.bass_utils import run_bass_kernel_spmd

    key = "full"
    if key not in _CACHE:
        _CACHE[key] = build_bass(n_cores=N_CORES, FC=1024)
    nc = _CACHE[key]

    in_maps = []
    for core in range(N_CORES):
        in_maps.append({
            "x": x[core].reshape(H, W * C),
            "wmix": wmix,
            "band": band,
        })
    res = run_bass_kernel_spmd(nc, in_maps, core_ids=list(range(N_CORES)),
                               trace=trace)
    out = np.stack([res.results[i]["y"].reshape(H, W, C) for i in range(N_CORES)], 0)
    return out, res


def kernel(x, co_matrix, w_spatial):
    return _run(x, co_matrix, w_spatial)[0]


def run_traced(x, co_matrix, w_spatial):
    _, res = _run(x, co_matrix, w_spatial, trace=True)
    return res.exec_time_ns
